# revision 6
# baseline (speedup 1.0000x reference)
"""DirectVoxGO render kernel for 8 Trainium2 NeuronCores — v2.

Host precomputes per-sample voxel index vi, trilinear weight pairs
(tpair, bf16) and segment masks; ships a supervoxel table G3 [V, 16]f32
(32 bf16 words per voxel, order [c][b][ch][a] so device folds read
contiguous halves). Device per 544-sample chunk:
  - batched indirect-DMA gathers (GB columns per instruction, one
    64B descriptor per sample, 3D dest AP [P, GB, 16])
  - trilinear fold: in-place V *= (tz x ty), two contiguous-half adds,
    in-place *= tx, pairwise fold -> [dens, r, g, b]
  - alpha path in product form via Rsqrt; segmented cumprod/cumsum via
    tensor_tensor_scan with reset masks; per-sample composited output
    to DRAM; host picks each ray's last sample.
"""
import numpy as np
import ml_dtypes

import concourse.bass as bass
import concourse.bacc as bacc
import concourse.tile as tile
from concourse import mybir
from concourse.bass_utils import run_bass_kernel_spmd

f32 = mybir.dt.float32
bf16 = mybir.dt.bfloat16
i32 = mybir.dt.int32
AF = mybir.ActivationFunctionType
OP = mybir.AluOpType

RES = 160
NCORES = 8
P = 128
K = 4224          # slots per partition row
L = 528           # chunk length; K % L == 0 -> 8 chunks
NCHUNK = K // L

TARGET_FILL = 4180
V = RES ** 3
ALPHA_INIT = 0.01
ACT_SHIFT = float(np.log(1.0 / (1.0 - ALPHA_INIT) - 1.0))

np_bf16 = ml_dtypes.bfloat16


# ----------------------------------------------------------------- host side

def build_layout(ray_id, n_rays):
    counts = np.bincount(ray_id, minlength=n_rays)
    nparts = NCORES * P
    part_of_ray = np.full(n_rays, -1, np.int64)
    start_of_ray = np.zeros(n_rays, np.int64)
    cur_p, fill = 0, 0
    for r in range(n_rays):
        c = counts[r]
        if c == 0:
            continue
        if fill + c > TARGET_FILL and fill > 0:
            cur_p += 1
            fill = 0
        assert cur_p < nparts, "ran out of partitions"
        assert fill + c <= K
        part_of_ray[r] = cur_p
        start_of_ray[r] = fill
        fill += c
    return counts, part_of_ray, start_of_ray


def host_prepare(xyz, density_grid, k0_grid, ray_id, n_rays):
    counts, part_of_ray, start_of_ray = build_layout(ray_id, n_rays)
    M = xyz.shape[0]
    nparts = NCORES * P

    ray_sample_start = np.concatenate([[0], np.cumsum(counts)[:-1]]).astype(np.int64)
    rid = ray_id.astype(np.int64)
    within = np.arange(M, dtype=np.int64) - ray_sample_start[rid]
    dest = part_of_ray[rid] * K + start_of_ray[rid] + within

    # per-sample voxel index + fraction pairs (f32 math matches reference)
    idx = xyz * np.float32(RES - 1)                       # [M,3] f32
    i0 = np.floor(idx)
    i0 = np.minimum(i0, np.float32(RES - 2)).astype(np.int32)
    i0 = np.maximum(i0, 0)
    f = idx - i0.astype(np.float32)                       # [M,3] f32
    visamp = (i0[:, 0].astype(np.int64) * (RES * RES)
              + i0[:, 1].astype(np.int64) * RES
              + i0[:, 2].astype(np.int64)).astype(np.int32)
    tp_samp = np.empty((M, 3, 2), np_bf16)
    tp_samp[:, :, 0] = (np.float32(1.0) - f).astype(np_bf16)
    tp_samp[:, :, 1] = f.astype(np_bf16)

    vip = np.zeros(nparts * K, np.int32)
    vip[dest] = visamp
    tpp = np.zeros((nparts * K, 3, 2), np_bf16)
    tpp[dest] = tp_samp
    m = np.ones(nparts * K, np.float32)
    valid = part_of_ray >= 0
    m[part_of_ray[valid] * K + start_of_ray[valid]] = 0.0
    mprod = np.float32(1.0) - m

    # per-ray output pick position
    p_global = part_of_ray[valid]
    core_of_ray = p_global // P
    p_local = p_global % P
    k_last = start_of_ray[valid] + counts[valid] - 1

    # supervoxel table: [V, 32] bf16, per-voxel word order [c][b][ch][a]
    grids = np.concatenate([density_grid, k0_grid], axis=0)
    g = np.ascontiguousarray(grids.astype(np_bf16))
    idxr = np.arange(RES)
    G3 = np.empty((RES, RES, RES, 2, 2, 4, 2), np_bf16)   # [x,y,z,c,b,ch,a]
    for a in range(2):
        xa = np.minimum(idxr + a, RES - 1)
        for b in range(2):
            yb = np.minimum(idxr + b, RES - 1)
            for c in range(2):
                zc = np.minimum(idxr + c, RES - 1)
                G3[:, :, :, c, b, :, a] = np.moveaxis(
                    g[:, xa][:, :, yb][:, :, :, zc], 0, -1)
    G3f = np.ascontiguousarray(G3.reshape(V, 32)).view(np.float32)  # [V, 16]

    meta = (np.where(valid)[0], core_of_ray, p_local, k_last)
    return (vip.reshape(NCORES, P, K),
            tpp.reshape(NCORES, P, K * 6),
            m.reshape(NCORES, P, K),
            mprod.reshape(NCORES, P, K),
            G3f, meta)


# --------------------------------------------------------------- bass kernel

def build_bass_program():
    nc = bacc.Bacc("TRN2", target_bir_lowering=False, debug=False,
                   num_devices=NCORES)

    vi_d = nc.dram_tensor("vip", [P, K], i32, kind="ExternalInput").ap()
    tp_d = nc.dram_tensor("tpp", [P, K * 6], bf16, kind="ExternalInput").ap()
    m_d = nc.dram_tensor("m", [P, K], f32, kind="ExternalInput").ap()
    mp_d = nc.dram_tensor("mprod", [P, K], f32, kind="ExternalInput").ap()
    g3_d = nc.dram_tensor("g3", [V, 16], f32, kind="ExternalInput").ap()
    out_d = nc.dram_tensor("outall", [P, K * 3], f32, kind="ExternalOutput").ap()

    with tile.TileContext(nc) as tc:
        io = tc.alloc_tile_pool(name="io", bufs=2)
        big = tc.alloc_tile_pool(name="big", bufs=2)
        mid = tc.alloc_tile_pool(name="mid", bufs=1)
        sc = tc.alloc_tile_pool(name="scan", bufs=2)
        oc = tc.alloc_tile_pool(name="oc", bufs=2)
        cpool = tc.alloc_tile_pool(name="const", bufs=1)
        shift_t = cpool.tile([P, 1], f32, tag="shift")
        nc.gpsimd.memset(shift_t[:], ACT_SHIFT)
        one_t = cpool.tile([P, 1], f32, tag="one")
        nc.gpsimd.memset(one_t[:], 1.0)

        prev_scan = None
        for j in range(NCHUNK):
            cs = j * L
            vi_t = io.tile([P, L], i32, tag="vi")
            nc.sync.dma_start(vi_t[:], vi_d[:, cs:cs + L])
            tp_t = io.tile([P, L * 6], bf16, tag="tp")
            nc.sync.dma_start(tp_t[:], tp_d[:, cs * 6:(cs + L) * 6])
            m_t = io.tile([P, L], f32, tag="m")
            nc.sync.dma_start(m_t[:], m_d[:, cs:cs + L])
            mp_t = io.tile([P, L], f32, tag="mp")
            nc.sync.dma_start(mp_t[:], mp_d[:, cs:cs + L])

            # ---- gather supervoxels: one 64B descriptor per sample
            V_t = big.tile([P, L * 16], f32, tag="V")
            for k in range(L):
                nc.gpsimd.indirect_dma_start(
                    out=V_t[:, k * 16:(k + 1) * 16], out_offset=None,
                    in_=g3_d,
                    in_offset=bass.IndirectOffsetOnAxis(
                        ap=vi_t[:, k:k + 1], axis=0))
            Vb = V_t[:].bitcast(bf16)            # [P, L*32]: [l][c][b][ch][a]

            tpv = tp_t[:].rearrange("p (l x w) -> p l x w", x=3, w=2)
            tx = tpv[:, :, 0]                    # [P, L, 2]
            ty = tpv[:, :, 1]
            tz = tpv[:, :, 2]

            # ---- wzy[l, c, b] = tz[c] * ty[b]
            wzy = mid.tile([P, L * 4], bf16, tag="wzy")
            nc.vector.tensor_tensor(
                out=wzy[:].rearrange("p (l c b) -> p l c b", c=2, b=2),
                in0=tz.unsqueeze(3).broadcast_to([P, L, 2, 2]),
                in1=ty.unsqueeze(2).broadcast_to([P, L, 2, 2]),
                op=OP.mult)

            # ---- in-place: V[l, cb, cha] *= wzy[l, cb]
            Vv = Vb.rearrange("p (l cb g) -> p l cb g", cb=4, g=8)
            nc.vector.tensor_tensor(
                out=Vv, in0=Vv,
                in1=wzy[:].rearrange("p (l cb) -> p l cb", cb=4)
                    .unsqueeze(3).broadcast_to([P, L, 4, 8]),
                op=OP.mult)

            # ---- fold c: contiguous halves of 32
            f1 = mid.tile([P, L * 16], bf16, tag="f1")
            Vh = Vb.rearrange("p (l h g) -> p l h g", h=2, g=16)
            nc.vector.tensor_tensor(
                out=f1[:].rearrange("p (l g) -> p l g", g=16),
                in0=Vh[:, :, 0], in1=Vh[:, :, 1], op=OP.add)
            # ---- fold b: contiguous halves of 16
            f2 = mid.tile([P, L * 8], bf16, tag="f2")
            f1h = f1[:].rearrange("p (l h g) -> p l h g", h=2, g=8)
            nc.vector.tensor_tensor(
                out=f2[:].rearrange("p (l g) -> p l g", g=8),
                in0=f1h[:, :, 0], in1=f1h[:, :, 1], op=OP.add)
            # ---- in-place: f2[l, ch, a] *= tx[l, a]
            f2v = f2[:].rearrange("p (l ch a) -> p l ch a", ch=4, a=2)
            nc.vector.tensor_tensor(
                out=f2v, in0=f2v,
                in1=tx.unsqueeze(2).broadcast_to([P, L, 4, 2]),
                op=OP.mult)
            # ---- fold a -> out4 [l, (dens, r, g, b)] f32
            out4 = mid.tile([P, L * 4], f32, tag="out4")
            out4v = out4[:].rearrange("p (l g) -> p l g", g=4)
            nc.vector.tensor_tensor(
                out=out4v, in0=f2v[:, :, :, 0], in1=f2v[:, :, :, 1],
                op=OP.add)

            # ---- alpha path: e = exp(d+shift); u = (1+e)^-1/2
            # r = sqrt(1+e) - 1 = (1+e)*u - 1;  weight = cumprod(u)*r
            e_t = mid.tile([P, L], f32, tag="e")
            nc.scalar.activation(e_t[:], out4v[:, :, 0], AF.Exp,
                                 bias=shift_t[:])
            r2_t = mid.tile([P, L], f32, tag="r2")
            nc.scalar.activation(r2_t[:], e_t[:], AF.Sqrt, bias=one_t[:])
            u_t = mid.tile([P, L], f32, tag="u")
            nc.vector.reciprocal(u_t[:], r2_t[:])

            scan4 = sc.tile([P, L * 4], f32, tag="scan4")
            s4v = scan4[:].rearrange("p (l f) -> p l f", f=4)
            init_la = 0.0 if prev_scan is None else \
                prev_scan[:].rearrange("p (l f) -> p l f", f=4)[:, L - 1, 3:4]
            nc.vector.tensor_tensor_scan(
                out=s4v[:, :, 3], data0=mp_t[:], data1=u_t[:],
                initial=init_la, op0=OP.max, op1=OP.mult)

            wgt = mid.tile([P, L], f32, tag="wgt")
            nc.vector.scalar_tensor_tensor(
                wgt[:], r2_t[:], -1.0, s4v[:, :, 3],
                op0=OP.add, op1=OP.mult)

            rgbs = mid.tile([P, L * 3], f32, tag="rgbs")
            rgbsv = rgbs[:].rearrange("p (l c) -> p l c", c=3)
            nc.scalar.activation(rgbsv, out4v[:, :, 1:4], AF.Sigmoid)
            wrgb = mid.tile([P, L * 3], f32, tag="wrgb")
            wrgbv = wrgb[:].rearrange("p (l c) -> p l c", c=3)
            nc.vector.tensor_tensor(
                out=wrgbv, in0=rgbsv,
                in1=wgt[:].unsqueeze(2).broadcast_to([P, L, 3]), op=OP.mult)

            for ch in range(3):
                init_c = 0.0 if prev_scan is None else \
                    prev_scan[:].rearrange("p (l f) -> p l f", f=4)[:, L - 1, ch:ch + 1]
                nc.vector.tensor_tensor_scan(
                    out=s4v[:, :, ch], data0=m_t[:], data1=wrgbv[:, :, ch],
                    initial=init_c, op0=OP.mult, op1=OP.add)

            # composited per-sample output: rgb_scan + Pinc (bkgd=1)
            outc = oc.tile([P, L * 3], f32, tag="outc")
            nc.vector.tensor_tensor(
                out=outc[:].rearrange("p (l c) -> p l c", c=3),
                in0=s4v[:, :, 0:3],
                in1=s4v[:, :, 3:4].broadcast_to([P, L, 3]), op=OP.add)
            nc.sync.dma_start(out_d[:, cs * 3:(cs + L) * 3], outc[:])
            prev_scan = scan4

        for pool in (cpool, oc, sc, mid, big, io):
            pool.release()

    nc.compile()
    return nc


_NC_CACHE = None


def _get_program():
    global _NC_CACHE
    if _NC_CACHE is None:
        _NC_CACHE = build_bass_program()
    return _NC_CACHE


def _run(inputs, trace=False, trace_kwargs=None):
    xyz = np.asarray(inputs["xyz"], np.float32)
    dg = np.asarray(inputs["density_grid"], np.float32)
    kg = np.asarray(inputs["k0_grid"], np.float32)
    ray_id = np.asarray(inputs["ray_id"]).astype(np.int64)
    n_rays = int(np.asarray(inputs["n_rays"]))

    vip, tpp, m, mprod, G3f, meta = host_prepare(xyz, dg, kg, ray_id, n_rays)
    nc = _get_program()
    in_maps = [{"vip": vip[c], "tpp": tpp[c].view(np.uint16),
                "m": m[c], "mprod": mprod[c], "g3": G3f}
               for c in range(NCORES)]
    res = run_bass_kernel_spmd(nc, in_maps, list(range(NCORES)),
                               trace=trace, **(trace_kwargs or {}))

    final = np.full((n_rays, 3), 1.0, np.float32)
    ridx, core_of_ray, p_local, k_last = meta
    outs = np.stack([res.results[c]["outall"].reshape(P, K, 3)
                     for c in range(NCORES)])
    final[ridx] = outs[core_of_ray, p_local, k_last]
    return final, res


def kernel(**inputs) -> np.ndarray:
    out, _ = _run(inputs)
    return out


# revision 9
# speedup vs baseline: 1.0283x; 1.0283x over previous
"""DirectVoxGO render kernel for 8 Trainium2 NeuronCores.

Host precomputes per-sample voxel index vi, trilinear weight pairs
(tpair, bf16) and segment masks; packs rays onto 8*128 partition rows
with least-loaded-first binning (K=4104 slots); ships a supervoxel
table G3 [V, 16]f32 (32 bf16 words per voxel, order [c][b][ch][a] so
device folds read contiguous halves). Device per 513-sample chunk:
  - per-column indirect-DMA gathers ([P,1] offsets, one 64B descriptor
    per sample; HW consumes exactly one offset per partition per
    instruction, so this is the maximal legal batch)
  - trilinear fold: in-place V *= (tz x ty), two contiguous-half adds,
    in-place *= tx, pairwise fold -> [dens, r, g, b]
  - alpha path in product form (exp/sqrt on ACT, reciprocal on DVE);
    segmented cumprod/cumsum via tensor_tensor_scan with reset masks;
    per-sample composited output to DRAM; host picks each ray's last
    sample row (the segment totals) to assemble [n_rays, 3].
"""
import numpy as np
import ml_dtypes

import concourse.bass as bass
import concourse.bacc as bacc
import concourse.tile as tile
from concourse import mybir
from concourse.bass_utils import run_bass_kernel_spmd

f32 = mybir.dt.float32
bf16 = mybir.dt.bfloat16
i32 = mybir.dt.int32
AF = mybir.ActivationFunctionType
OP = mybir.AluOpType

RES = 160
NCORES = 8
P = 128
K = 4104          # slots per partition row
L = 513           # chunk length; K % L == 0 -> 8 chunks
NCHUNK = K // L
V = RES ** 3
ALPHA_INIT = 0.01
ACT_SHIFT = float(np.log(1.0 / (1.0 - ALPHA_INIT) - 1.0))

np_bf16 = ml_dtypes.bfloat16


# ----------------------------------------------------------------- host side

def build_layout(ray_id, n_rays):
    """Least-loaded-first bin packing of rays onto 1024 partition rows."""
    import heapq
    counts = np.bincount(ray_id, minlength=n_rays)
    nparts = NCORES * P
    order = np.argsort(-counts, kind="stable")
    part_of_ray = np.full(n_rays, -1, np.int64)
    start_of_ray = np.zeros(n_rays, np.int64)
    heap = [(0, p) for p in range(nparts)]
    heapq.heapify(heap)
    for r in order:
        c = int(counts[r])
        if c == 0:
            continue
        fill, p = heapq.heappop(heap)
        assert fill + c <= K, "partition overflow"
        part_of_ray[r] = p
        start_of_ray[r] = fill
        heapq.heappush(heap, (fill + c, p))
    return counts, part_of_ray, start_of_ray


def host_prepare(xyz, density_grid, k0_grid, ray_id, n_rays):
    counts, part_of_ray, start_of_ray = build_layout(ray_id, n_rays)
    M = xyz.shape[0]
    nparts = NCORES * P

    ray_sample_start = np.concatenate([[0], np.cumsum(counts)[:-1]]).astype(np.int64)
    rid = ray_id.astype(np.int64)
    within = np.arange(M, dtype=np.int64) - ray_sample_start[rid]
    dest = part_of_ray[rid] * K + start_of_ray[rid] + within

    # per-sample voxel index + fraction pairs (f32 math matches reference)
    idx = xyz * np.float32(RES - 1)                       # [M,3] f32
    i0 = np.floor(idx)
    i0 = np.minimum(i0, np.float32(RES - 2)).astype(np.int32)
    i0 = np.maximum(i0, 0)
    f = idx - i0.astype(np.float32)                       # [M,3] f32
    visamp = (i0[:, 0].astype(np.int64) * (RES * RES)
              + i0[:, 1].astype(np.int64) * RES
              + i0[:, 2].astype(np.int64)).astype(np.int32)
    tp_samp = np.empty((M, 3, 2), np_bf16)
    tp_samp[:, :, 0] = (np.float32(1.0) - f).astype(np_bf16)
    tp_samp[:, :, 1] = f.astype(np_bf16)

    vip = np.zeros(nparts * K, np.int32)
    vip[dest] = visamp
    tpp = np.zeros((nparts * K, 3, 2), np_bf16)
    tpp[dest] = tp_samp
    m = np.ones(nparts * K, np.float32)
    valid = part_of_ray >= 0
    m[part_of_ray[valid] * K + start_of_ray[valid]] = 0.0
    mprod = np.float32(1.0) - m

    # per-ray output pick position
    p_global = part_of_ray[valid]
    core_of_ray = p_global // P
    p_local = p_global % P
    k_last = start_of_ray[valid] + counts[valid] - 1

    # supervoxel table: [V, 32] bf16, per-voxel word order [c][b][ch][a]
    grids = np.concatenate([density_grid, k0_grid], axis=0)
    g = np.ascontiguousarray(grids.astype(np_bf16))
    idxr = np.arange(RES)
    G3 = np.empty((RES, RES, RES, 2, 2, 4, 2), np_bf16)   # [x,y,z,c,b,ch,a]
    for a in range(2):
        xa = np.minimum(idxr + a, RES - 1)
        for b in range(2):
            yb = np.minimum(idxr + b, RES - 1)
            for c in range(2):
                zc = np.minimum(idxr + c, RES - 1)
                G3[:, :, :, c, b, :, a] = np.moveaxis(
                    g[:, xa][:, :, yb][:, :, :, zc], 0, -1)
    G3f = np.ascontiguousarray(G3.reshape(V, 32)).view(np.float32)  # [V, 16]

    meta = (np.where(valid)[0], core_of_ray, p_local, k_last)
    return (vip.reshape(NCORES, P, K),
            tpp.reshape(NCORES, P, K * 6),
            m.reshape(NCORES, P, K),
            mprod.reshape(NCORES, P, K),
            G3f, meta)


# --------------------------------------------------------------- bass kernel

def build_bass_program():
    nc = bacc.Bacc("TRN2", target_bir_lowering=False, debug=False,
                   num_devices=NCORES)

    vi_d = nc.dram_tensor("vip", [P, K], i32, kind="ExternalInput").ap()
    tp_d = nc.dram_tensor("tpp", [P, K * 6], bf16, kind="ExternalInput").ap()
    m_d = nc.dram_tensor("m", [P, K], f32, kind="ExternalInput").ap()
    mp_d = nc.dram_tensor("mprod", [P, K], f32, kind="ExternalInput").ap()
    g3_d = nc.dram_tensor("g3", [V, 16], f32, kind="ExternalInput").ap()
    out_d = nc.dram_tensor("outall", [P, K * 3], f32, kind="ExternalOutput").ap()

    with tile.TileContext(nc) as tc:
        io = tc.alloc_tile_pool(name="io", bufs=2)
        big = tc.alloc_tile_pool(name="big", bufs=2)
        mid = tc.alloc_tile_pool(name="mid", bufs=1)
        sc = tc.alloc_tile_pool(name="scan", bufs=2)
        oc = tc.alloc_tile_pool(name="oc", bufs=2)
        cpool = tc.alloc_tile_pool(name="const", bufs=1)
        shift_t = cpool.tile([P, 1], f32, tag="shift")
        nc.gpsimd.memset(shift_t[:], ACT_SHIFT)
        one_t = cpool.tile([P, 1], f32, tag="one")
        nc.gpsimd.memset(one_t[:], 1.0)

        prev_scan = None
        for j in range(NCHUNK):
            cs = j * L
            vi_t = io.tile([P, L], i32, tag="vi")
            nc.sync.dma_start(vi_t[:], vi_d[:, cs:cs + L])
            tp_t = io.tile([P, L * 6], bf16, tag="tp")
            nc.sync.dma_start(tp_t[:], tp_d[:, cs * 6:(cs + L) * 6])
            m_t = io.tile([P, L], f32, tag="m")
            nc.sync.dma_start(m_t[:], m_d[:, cs:cs + L])
            mp_t = io.tile([P, L], f32, tag="mp")
            nc.sync.dma_start(mp_t[:], mp_d[:, cs:cs + L])

            # ---- gather supervoxels: one 64B descriptor per sample
            V_t = big.tile([P, L * 16], f32, tag="V")
            for k in range(L):
                nc.gpsimd.indirect_dma_start(
                    out=V_t[:, k * 16:(k + 1) * 16], out_offset=None,
                    in_=g3_d,
                    in_offset=bass.IndirectOffsetOnAxis(
                        ap=vi_t[:, k:k + 1], axis=0))
            Vb = V_t[:].bitcast(bf16)            # [P, L*32]: [l][c][b][ch][a]

            tpv = tp_t[:].rearrange("p (l x w) -> p l x w", x=3, w=2)
            tx = tpv[:, :, 0]                    # [P, L, 2]
            ty = tpv[:, :, 1]
            tz = tpv[:, :, 2]

            # ---- wzy[l, c, b] = tz[c] * ty[b]
            wzy = mid.tile([P, L * 4], bf16, tag="wzy")
            nc.vector.tensor_tensor(
                out=wzy[:].rearrange("p (l c b) -> p l c b", c=2, b=2),
                in0=tz.unsqueeze(3).broadcast_to([P, L, 2, 2]),
                in1=ty.unsqueeze(2).broadcast_to([P, L, 2, 2]),
                op=OP.mult)

            # ---- in-place: V[l, cb, cha] *= wzy[l, cb]
            Vv = Vb.rearrange("p (l cb g) -> p l cb g", cb=4, g=8)
            nc.vector.tensor_tensor(
                out=Vv, in0=Vv,
                in1=wzy[:].rearrange("p (l cb) -> p l cb", cb=4)
                    .unsqueeze(3).broadcast_to([P, L, 4, 8]),
                op=OP.mult)

            # ---- fold c: contiguous halves of 32
            f1 = mid.tile([P, L * 16], bf16, tag="f1")
            Vh = Vb.rearrange("p (l h g) -> p l h g", h=2, g=16)
            nc.vector.tensor_tensor(
                out=f1[:].rearrange("p (l g) -> p l g", g=16),
                in0=Vh[:, :, 0], in1=Vh[:, :, 1], op=OP.add)
            # ---- fold b: contiguous halves of 16
            f2 = mid.tile([P, L * 8], bf16, tag="f2")
            f1h = f1[:].rearrange("p (l h g) -> p l h g", h=2, g=8)
            nc.vector.tensor_tensor(
                out=f2[:].rearrange("p (l g) -> p l g", g=8),
                in0=f1h[:, :, 0], in1=f1h[:, :, 1], op=OP.add)
            # ---- in-place: f2[l, ch, a] *= tx[l, a]
            f2v = f2[:].rearrange("p (l ch a) -> p l ch a", ch=4, a=2)
            nc.vector.tensor_tensor(
                out=f2v, in0=f2v,
                in1=tx.unsqueeze(2).broadcast_to([P, L, 4, 2]),
                op=OP.mult)
            # ---- fold a -> out4 [l, (dens, r, g, b)] f32
            out4 = mid.tile([P, L * 4], f32, tag="out4")
            out4v = out4[:].rearrange("p (l g) -> p l g", g=4)
            nc.vector.tensor_tensor(
                out=out4v, in0=f2v[:, :, :, 0], in1=f2v[:, :, :, 1],
                op=OP.add)

            # ---- alpha path: e = exp(d+shift); u = (1+e)^-1/2
            # r = sqrt(1+e) - 1 = (1+e)*u - 1;  weight = cumprod(u)*r
            e_t = mid.tile([P, L], f32, tag="e")
            nc.scalar.activation(e_t[:], out4v[:, :, 0], AF.Exp,
                                 bias=shift_t[:])
            r2_t = mid.tile([P, L], f32, tag="r2")
            nc.scalar.activation(r2_t[:], e_t[:], AF.Sqrt, bias=one_t[:])
            u_t = mid.tile([P, L], f32, tag="u")
            nc.vector.reciprocal(u_t[:], r2_t[:])

            scan4 = sc.tile([P, L * 4], f32, tag="scan4")
            s4v = scan4[:].rearrange("p (l f) -> p l f", f=4)
            init_la = 0.0 if prev_scan is None else \
                prev_scan[:].rearrange("p (l f) -> p l f", f=4)[:, L - 1, 3:4]
            nc.vector.tensor_tensor_scan(
                out=s4v[:, :, 3], data0=mp_t[:], data1=u_t[:],
                initial=init_la, op0=OP.max, op1=OP.mult)

            wgt = mid.tile([P, L], f32, tag="wgt")
            nc.vector.scalar_tensor_tensor(
                wgt[:], r2_t[:], -1.0, s4v[:, :, 3],
                op0=OP.add, op1=OP.mult)

            rgbs = mid.tile([P, L * 3], f32, tag="rgbs")
            rgbsv = rgbs[:].rearrange("p (l c) -> p l c", c=3)
            nc.scalar.activation(rgbsv, out4v[:, :, 1:4], AF.Sigmoid)
            wrgb = mid.tile([P, L * 3], f32, tag="wrgb")
            wrgbv = wrgb[:].rearrange("p (l c) -> p l c", c=3)
            nc.vector.tensor_tensor(
                out=wrgbv, in0=rgbsv,
                in1=wgt[:].unsqueeze(2).broadcast_to([P, L, 3]), op=OP.mult)

            for ch in range(3):
                init_c = 0.0 if prev_scan is None else \
                    prev_scan[:].rearrange("p (l f) -> p l f", f=4)[:, L - 1, ch:ch + 1]
                nc.vector.tensor_tensor_scan(
                    out=s4v[:, :, ch], data0=m_t[:], data1=wrgbv[:, :, ch],
                    initial=init_c, op0=OP.mult, op1=OP.add)

            # composited per-sample output: rgb_scan + Pinc (bkgd=1)
            outc = oc.tile([P, L * 3], f32, tag="outc")
            nc.vector.tensor_tensor(
                out=outc[:].rearrange("p (l c) -> p l c", c=3),
                in0=s4v[:, :, 0:3],
                in1=s4v[:, :, 3:4].broadcast_to([P, L, 3]), op=OP.add)
            nc.sync.dma_start(out_d[:, cs * 3:(cs + L) * 3], outc[:])
            prev_scan = scan4

        for pool in (cpool, oc, sc, mid, big, io):
            pool.release()

    nc.compile()
    return nc


_NC_CACHE = None


def _get_program():
    global _NC_CACHE
    if _NC_CACHE is None:
        _NC_CACHE = build_bass_program()
    return _NC_CACHE


def _run(inputs, trace=False, trace_kwargs=None):
    xyz = np.asarray(inputs["xyz"], np.float32)
    dg = np.asarray(inputs["density_grid"], np.float32)
    kg = np.asarray(inputs["k0_grid"], np.float32)
    ray_id = np.asarray(inputs["ray_id"]).astype(np.int64)
    n_rays = int(np.asarray(inputs["n_rays"]))

    vip, tpp, m, mprod, G3f, meta = host_prepare(xyz, dg, kg, ray_id, n_rays)
    nc = _get_program()
    in_maps = [{"vip": vip[c], "tpp": tpp[c].view(np.uint16),
                "m": m[c], "mprod": mprod[c], "g3": G3f}
               for c in range(NCORES)]
    res = run_bass_kernel_spmd(nc, in_maps, list(range(NCORES)),
                               trace=trace, **(trace_kwargs or {}))

    final = np.full((n_rays, 3), 1.0, np.float32)
    ridx, core_of_ray, p_local, k_last = meta
    outs = np.stack([res.results[c]["outall"].reshape(P, K, 3)
                     for c in range(NCORES)])
    final[ridx] = outs[core_of_ray, p_local, k_last]
    return final, res


def kernel(**inputs) -> np.ndarray:
    out, _ = _run(inputs)
    return out


# revision 10
# speedup vs baseline: 1.0401x; 1.0115x over previous
"""DirectVoxGO render kernel for 8 Trainium2 NeuronCores.

Host precomputes per-sample voxel index vi, trilinear weight pairs
(tpair, bf16) and segment masks; packs rays onto 8*128 partition rows
with least-loaded-first binning (K=4104 slots); ships a supervoxel
table G3 [V, 16]f32 (32 bf16 words per voxel, order [c][b][ch][a] so
device folds read contiguous halves). Device per 513-sample chunk:
  - per-column indirect-DMA gathers ([P,1] offsets, one 64B descriptor
    per sample; HW consumes exactly one offset per partition per
    instruction, so this is the maximal legal batch)
  - trilinear fold: in-place V *= (tz x ty), two contiguous-half adds,
    in-place *= tx, pairwise fold -> [dens, r, g, b]
  - alpha path in product form (exp/sqrt on ACT, reciprocal on DVE);
    segmented cumprod/cumsum via tensor_tensor_scan with reset masks;
    per-sample composited output to DRAM; host picks each ray's last
    sample row (the segment totals) to assemble [n_rays, 3].
"""
import numpy as np
import ml_dtypes

import concourse.bass as bass
import concourse.bacc as bacc
import concourse.tile as tile
from concourse import mybir
from concourse.bass_utils import run_bass_kernel_spmd

f32 = mybir.dt.float32
bf16 = mybir.dt.bfloat16
i32 = mybir.dt.int32
AF = mybir.ActivationFunctionType
OP = mybir.AluOpType

RES = 160
NCORES = 8
P = 128
K = 4104          # slots per partition row
L = 171           # chunk length; K % L == 0 -> 24 chunks
NCHUNK = K // L
V = RES ** 3
ALPHA_INIT = 0.01
ACT_SHIFT = float(np.log(1.0 / (1.0 - ALPHA_INIT) - 1.0))

np_bf16 = ml_dtypes.bfloat16


# ----------------------------------------------------------------- host side

def build_layout(ray_id, n_rays):
    """Least-loaded-first bin packing of rays onto 1024 partition rows."""
    import heapq
    counts = np.bincount(ray_id, minlength=n_rays)
    nparts = NCORES * P
    order = np.argsort(-counts, kind="stable")
    part_of_ray = np.full(n_rays, -1, np.int64)
    start_of_ray = np.zeros(n_rays, np.int64)
    heap = [(0, p) for p in range(nparts)]
    heapq.heapify(heap)
    for r in order:
        c = int(counts[r])
        if c == 0:
            continue
        fill, p = heapq.heappop(heap)
        assert fill + c <= K, "partition overflow"
        part_of_ray[r] = p
        start_of_ray[r] = fill
        heapq.heappush(heap, (fill + c, p))
    return counts, part_of_ray, start_of_ray


def host_prepare(xyz, density_grid, k0_grid, ray_id, n_rays):
    counts, part_of_ray, start_of_ray = build_layout(ray_id, n_rays)
    M = xyz.shape[0]
    nparts = NCORES * P

    ray_sample_start = np.concatenate([[0], np.cumsum(counts)[:-1]]).astype(np.int64)
    rid = ray_id.astype(np.int64)
    within = np.arange(M, dtype=np.int64) - ray_sample_start[rid]
    dest = part_of_ray[rid] * K + start_of_ray[rid] + within

    # per-sample voxel index + fraction pairs (f32 math matches reference)
    idx = xyz * np.float32(RES - 1)                       # [M,3] f32
    i0 = np.floor(idx)
    i0 = np.minimum(i0, np.float32(RES - 2)).astype(np.int32)
    i0 = np.maximum(i0, 0)
    f = idx - i0.astype(np.float32)                       # [M,3] f32
    visamp = (i0[:, 0].astype(np.int64) * (RES * RES)
              + i0[:, 1].astype(np.int64) * RES
              + i0[:, 2].astype(np.int64)).astype(np.int32)
    tp_samp = np.empty((M, 3, 2), np_bf16)
    tp_samp[:, :, 0] = (np.float32(1.0) - f).astype(np_bf16)
    tp_samp[:, :, 1] = f.astype(np_bf16)

    vip = np.zeros(nparts * K, np.int32)
    vip[dest] = visamp
    tpp = np.zeros((nparts * K, 3, 2), np_bf16)
    tpp[dest] = tp_samp
    m = np.ones(nparts * K, np.float32)
    valid = part_of_ray >= 0
    m[part_of_ray[valid] * K + start_of_ray[valid]] = 0.0
    mprod = np.float32(1.0) - m

    # per-ray output pick position
    p_global = part_of_ray[valid]
    core_of_ray = p_global // P
    p_local = p_global % P
    k_last = start_of_ray[valid] + counts[valid] - 1

    # supervoxel table: [V, 32] bf16, per-voxel word order [c][b][ch][a]
    grids = np.concatenate([density_grid, k0_grid], axis=0)
    g = np.ascontiguousarray(grids.astype(np_bf16))
    idxr = np.arange(RES)
    G3 = np.empty((RES, RES, RES, 2, 2, 4, 2), np_bf16)   # [x,y,z,c,b,ch,a]
    for a in range(2):
        xa = np.minimum(idxr + a, RES - 1)
        for b in range(2):
            yb = np.minimum(idxr + b, RES - 1)
            for c in range(2):
                zc = np.minimum(idxr + c, RES - 1)
                G3[:, :, :, c, b, :, a] = np.moveaxis(
                    g[:, xa][:, :, yb][:, :, :, zc], 0, -1)
    G3f = np.ascontiguousarray(G3.reshape(V, 32)).view(np.float32)  # [V, 16]

    meta = (np.where(valid)[0], core_of_ray, p_local, k_last)
    return (vip.reshape(NCORES, P, K),
            tpp.reshape(NCORES, P, K * 6),
            m.reshape(NCORES, P, K),
            mprod.reshape(NCORES, P, K),
            G3f, meta)


# --------------------------------------------------------------- bass kernel

def build_bass_program():
    nc = bacc.Bacc("TRN2", target_bir_lowering=False, debug=False,
                   num_devices=NCORES)

    vi_d = nc.dram_tensor("vip", [P, K], i32, kind="ExternalInput").ap()
    tp_d = nc.dram_tensor("tpp", [P, K * 6], bf16, kind="ExternalInput").ap()
    m_d = nc.dram_tensor("m", [P, K], f32, kind="ExternalInput").ap()
    mp_d = nc.dram_tensor("mprod", [P, K], f32, kind="ExternalInput").ap()
    g3_d = nc.dram_tensor("g3", [V, 16], f32, kind="ExternalInput").ap()
    out_d = nc.dram_tensor("outall", [P, K * 3], f32, kind="ExternalOutput").ap()

    with tile.TileContext(nc) as tc:
        io = tc.alloc_tile_pool(name="io", bufs=2)
        big = tc.alloc_tile_pool(name="big", bufs=2)
        mid = tc.alloc_tile_pool(name="mid", bufs=1)
        sc = tc.alloc_tile_pool(name="scan", bufs=2)
        oc = tc.alloc_tile_pool(name="oc", bufs=2)
        cpool = tc.alloc_tile_pool(name="const", bufs=1)
        shift_t = cpool.tile([P, 1], f32, tag="shift")
        nc.gpsimd.memset(shift_t[:], ACT_SHIFT)
        one_t = cpool.tile([P, 1], f32, tag="one")
        nc.gpsimd.memset(one_t[:], 1.0)

        prev_scan = None
        for j in range(NCHUNK):
            cs = j * L
            vi_t = io.tile([P, L], i32, tag="vi")
            nc.sync.dma_start(vi_t[:], vi_d[:, cs:cs + L])
            tp_t = io.tile([P, L * 6], bf16, tag="tp")
            nc.sync.dma_start(tp_t[:], tp_d[:, cs * 6:(cs + L) * 6])
            m_t = io.tile([P, L], f32, tag="m")
            nc.sync.dma_start(m_t[:], m_d[:, cs:cs + L])
            mp_t = io.tile([P, L], f32, tag="mp")
            nc.sync.dma_start(mp_t[:], mp_d[:, cs:cs + L])

            # ---- gather supervoxels: one 64B descriptor per sample
            V_t = big.tile([P, L * 16], f32, tag="V")
            for k in range(L):
                nc.gpsimd.indirect_dma_start(
                    out=V_t[:, k * 16:(k + 1) * 16], out_offset=None,
                    in_=g3_d,
                    in_offset=bass.IndirectOffsetOnAxis(
                        ap=vi_t[:, k:k + 1], axis=0))
            Vb = V_t[:].bitcast(bf16)            # [P, L*32]: [l][c][b][ch][a]

            tpv = tp_t[:].rearrange("p (l x w) -> p l x w", x=3, w=2)
            tx = tpv[:, :, 0]                    # [P, L, 2]
            ty = tpv[:, :, 1]
            tz = tpv[:, :, 2]

            # ---- wzy[l, c, b] = tz[c] * ty[b]
            wzy = mid.tile([P, L * 4], bf16, tag="wzy")
            nc.vector.tensor_tensor(
                out=wzy[:].rearrange("p (l c b) -> p l c b", c=2, b=2),
                in0=tz.unsqueeze(3).broadcast_to([P, L, 2, 2]),
                in1=ty.unsqueeze(2).broadcast_to([P, L, 2, 2]),
                op=OP.mult)

            # ---- in-place: V[l, cb, cha] *= wzy[l, cb]
            Vv = Vb.rearrange("p (l cb g) -> p l cb g", cb=4, g=8)
            nc.vector.tensor_tensor(
                out=Vv, in0=Vv,
                in1=wzy[:].rearrange("p (l cb) -> p l cb", cb=4)
                    .unsqueeze(3).broadcast_to([P, L, 4, 8]),
                op=OP.mult)

            # ---- fold c: contiguous halves of 32
            f1 = mid.tile([P, L * 16], bf16, tag="f1")
            Vh = Vb.rearrange("p (l h g) -> p l h g", h=2, g=16)
            nc.vector.tensor_tensor(
                out=f1[:].rearrange("p (l g) -> p l g", g=16),
                in0=Vh[:, :, 0], in1=Vh[:, :, 1], op=OP.add)
            # ---- fold b: contiguous halves of 16
            f2 = mid.tile([P, L * 8], bf16, tag="f2")
            f1h = f1[:].rearrange("p (l h g) -> p l h g", h=2, g=8)
            nc.vector.tensor_tensor(
                out=f2[:].rearrange("p (l g) -> p l g", g=8),
                in0=f1h[:, :, 0], in1=f1h[:, :, 1], op=OP.add)
            # ---- in-place: f2[l, ch, a] *= tx[l, a]
            f2v = f2[:].rearrange("p (l ch a) -> p l ch a", ch=4, a=2)
            nc.vector.tensor_tensor(
                out=f2v, in0=f2v,
                in1=tx.unsqueeze(2).broadcast_to([P, L, 4, 2]),
                op=OP.mult)
            # ---- fold a -> out4 [l, (dens, r, g, b)] f32
            out4 = mid.tile([P, L * 4], f32, tag="out4")
            out4v = out4[:].rearrange("p (l g) -> p l g", g=4)
            nc.vector.tensor_tensor(
                out=out4v, in0=f2v[:, :, :, 0], in1=f2v[:, :, :, 1],
                op=OP.add)

            # ---- alpha path: e = exp(d+shift); u = (1+e)^-1/2
            # r = sqrt(1+e) - 1 = (1+e)*u - 1;  weight = cumprod(u)*r
            e_t = mid.tile([P, L], f32, tag="e")
            nc.scalar.activation(e_t[:], out4v[:, :, 0], AF.Exp,
                                 bias=shift_t[:])
            r2_t = mid.tile([P, L], f32, tag="r2")
            nc.scalar.activation(r2_t[:], e_t[:], AF.Sqrt, bias=one_t[:])
            u_t = mid.tile([P, L], f32, tag="u")
            nc.vector.reciprocal(u_t[:], r2_t[:])

            scan4 = sc.tile([P, L * 4], f32, tag="scan4")
            s4v = scan4[:].rearrange("p (l f) -> p l f", f=4)
            init_la = 0.0 if prev_scan is None else \
                prev_scan[:].rearrange("p (l f) -> p l f", f=4)[:, L - 1, 3:4]
            nc.vector.tensor_tensor_scan(
                out=s4v[:, :, 3], data0=mp_t[:], data1=u_t[:],
                initial=init_la, op0=OP.max, op1=OP.mult)

            wgt = mid.tile([P, L], f32, tag="wgt")
            nc.vector.scalar_tensor_tensor(
                wgt[:], r2_t[:], -1.0, s4v[:, :, 3],
                op0=OP.add, op1=OP.mult)

            rgbs = mid.tile([P, L * 3], f32, tag="rgbs")
            rgbsv = rgbs[:].rearrange("p (l c) -> p l c", c=3)
            nc.scalar.activation(rgbsv, out4v[:, :, 1:4], AF.Sigmoid)
            wrgb = mid.tile([P, L * 3], f32, tag="wrgb")
            wrgbv = wrgb[:].rearrange("p (l c) -> p l c", c=3)
            nc.vector.tensor_tensor(
                out=wrgbv, in0=rgbsv,
                in1=wgt[:].unsqueeze(2).broadcast_to([P, L, 3]), op=OP.mult)

            for ch in range(3):
                init_c = 0.0 if prev_scan is None else \
                    prev_scan[:].rearrange("p (l f) -> p l f", f=4)[:, L - 1, ch:ch + 1]
                nc.vector.tensor_tensor_scan(
                    out=s4v[:, :, ch], data0=m_t[:], data1=wrgbv[:, :, ch],
                    initial=init_c, op0=OP.mult, op1=OP.add)

            # composited per-sample output: rgb_scan + Pinc (bkgd=1)
            outc = oc.tile([P, L * 3], f32, tag="outc")
            nc.vector.tensor_tensor(
                out=outc[:].rearrange("p (l c) -> p l c", c=3),
                in0=s4v[:, :, 0:3],
                in1=s4v[:, :, 3:4].broadcast_to([P, L, 3]), op=OP.add)
            nc.sync.dma_start(out_d[:, cs * 3:(cs + L) * 3], outc[:])
            prev_scan = scan4

        for pool in (cpool, oc, sc, mid, big, io):
            pool.release()

    nc.compile()
    return nc


_NC_CACHE = None


def _get_program():
    global _NC_CACHE
    if _NC_CACHE is None:
        _NC_CACHE = build_bass_program()
    return _NC_CACHE


def _run(inputs, trace=False, trace_kwargs=None):
    xyz = np.asarray(inputs["xyz"], np.float32)
    dg = np.asarray(inputs["density_grid"], np.float32)
    kg = np.asarray(inputs["k0_grid"], np.float32)
    ray_id = np.asarray(inputs["ray_id"]).astype(np.int64)
    n_rays = int(np.asarray(inputs["n_rays"]))

    vip, tpp, m, mprod, G3f, meta = host_prepare(xyz, dg, kg, ray_id, n_rays)
    nc = _get_program()
    in_maps = [{"vip": vip[c], "tpp": tpp[c].view(np.uint16),
                "m": m[c], "mprod": mprod[c], "g3": G3f}
               for c in range(NCORES)]
    res = run_bass_kernel_spmd(nc, in_maps, list(range(NCORES)),
                               trace=trace, **(trace_kwargs or {}))

    final = np.full((n_rays, 3), 1.0, np.float32)
    ridx, core_of_ray, p_local, k_last = meta
    outs = np.stack([res.results[c]["outall"].reshape(P, K, 3)
                     for c in range(NCORES)])
    final[ridx] = outs[core_of_ray, p_local, k_last]
    return final, res


def kernel(**inputs) -> np.ndarray:
    out, _ = _run(inputs)
    return out
